# revision 17
# baseline (speedup 1.0000x reference)
"""Trainium2 Bass kernel for nn_BasicBlock (binary-conv basic block).

Forward semantics of the reference collapse to:
  a    = sign(x + bias1[b,c]),  bias1 = silu(emb) @ m1w.T + m1b
  S    = conv3x3(a, sign(conv_w))                (integer-valued sum)
  z    = A[o]*S + C[b,o] + 0.5*(x[2o]+x[2o+1])   (BN + residual pool + bias2)
         where A = mean|conv_w[o]| * gamma/sqrt(var+eps)
               C = (conv_b-mean)*gamma/sqrt(var+eps)+beta+bias2
  out  = alpha*z + beta*|z| + C3                 (PReLU + bias3)
         alpha=(1+a)/2, beta=(1-a)/2, C3 = bias3

Sharding: data-parallel over batch, 2 images per core on 8 cores.

The conv runs as 9 shifted fp8 DoubleRow matmuls (weights/activations are
exactly representable in fp8e4; accumulation is fp32 in PSUM, so the conv
sum is exact).  The channel-pair mean-pool residual is folded into the same
PSUM accumulation as two fp32 matmuls with weights 0.5/A[m] placed on the
channel-pair diagonal.  Activations live in a 66-wide zero-padded row
layout so each conv tap is a single stride-1 AP; the 2/66 junk columns are
computed and discarded at DMA-out.
"""

import re

import numpy as np

import concourse.bass as bass
import concourse.mybir as mybir
import concourse.tile as tile
from concourse.vector_clock import ScopedClock, VectorClock

F32 = mybir.dt.float32
BF16 = mybir.dt.bfloat16
F8 = mybir.dt.float8e4
AF = mybir.ActivationFunctionType
ALU = mybir.AluOpType
DR = mybir.MatmulPerfMode.DoubleRow

N_CORES = 8
B_LOC = 2           # images per core
CIN = 256
COUT = 128
H = W = 64
PW = 66             # padded row width (1 left + 1 right)
NPAD = PW * PW      # 4356 padded cells (66 rows x 66 cols)
JSTR = 4368         # j-block stride, padded to a multiple of 16
QSPAN = (H - 1) * PW + W  # 4222 output positions in padded coords
E = 512

# chunk list over the Q span: 8 x 512 + 1 x 126
CHUNKS = [(i * 512, 512) for i in range(8)] + [(4096, QSPAN - 4096)]
GROUPS = [[0, 1, 2], [3, 4, 5], [6, 7, 8]]

# Use the parametric-relu ACT table entry for the epilogue when True;
# fall back to the alpha*z + beta*|z| identity (Abs) when False.
USE_PRELU = False
DEBUG = False


def _patch_tile_drain():
    """walrus in this container only accepts one sync-wait per Drain; split
    the kernel-tail drain's waits across one drain per logical processor."""
    if getattr(tile.TileContext, "_drain_split_patched", False):
        return

    def _drain_and_barrier(self, tick_clock, wait_clock):
        vals = [int(s) for s in re.findall(r"-?\d+", repr(tick_clock.global_clock))]
        for i, v in enumerate(vals):
            if v > 0:
                part = VectorClock()
                part.require_at_least(i, v)
                d = self.nc.sync.drain()
                wait_clock.add_sem_waits(d.ins, ScopedClock({None: part}))
        self.nc.sync.drain()
        self.nc.all_engine_barrier()
        assert self.sems is not None
        popped = self.nc._tile_sem_poison_stack.pop()
        assert popped is self._sem_poison
        self.nc.clear_and_free_semaphores(list(self.sems.allocated().values()))
        self.nc.all_engine_barrier()

    tile.TileContext._drain_and_barrier = _drain_and_barrier
    tile.TileContext._drain_split_patched = True


def _split_multi_waits(nc):
    """This container's walrus accepts at most ONE sync-wait per instruction.
    Hoist extra waits onto no-op instructions injected just before, on the
    same engine (the engine executes the nop waits first, preserving order)."""
    f = nc.m.functions[0]
    for bb in f.blocks:
        out = []
        changed = False
        for inst in bb.instructions:
            si = inst.sync_info
            if si is not None and si.on_wait and len(si.on_wait) > 1:
                waits = list(si.on_wait)
                for w in waits[:-1]:
                    nop = mybir.InstNoOp(
                        name=nc.get_next_instruction_name(),
                        sync_info=mybir.SyncInfo(on_wait=[w], on_update=[]),
                        bass_nofuse=True,
                        engine=inst.engine,
                    )
                    out.append(nop)
                si.on_wait = [waits[-1]]
                inst.sync_info = si
                changed = True
            out.append(inst)
        if changed:
            bb.instructions = out


def build_program(split_waits=True):
    _patch_tile_drain()
    nc = bass.Bass()

    x_d = nc.declare_dram_parameter("x", [B_LOC, CIN, H, W], F32, isOutput=False)
    emb_d = nc.declare_dram_parameter("emb", [B_LOC, E], F32, isOutput=False)
    m1w_d = nc.declare_dram_parameter("move1_w", [CIN, E], F32, isOutput=False)
    m1b_d = nc.declare_dram_parameter("move1_b", [CIN], F32, isOutput=False)
    cw_d = nc.declare_dram_parameter("conv_w", [COUT, CIN, 3, 3], F32, isOutput=False)
    cb_d = nc.declare_dram_parameter("conv_b", [COUT], F32, isOutput=False)
    gam_d = nc.declare_dram_parameter("bn_gamma", [COUT], F32, isOutput=False)
    bet_d = nc.declare_dram_parameter("bn_beta", [COUT], F32, isOutput=False)
    mu_d = nc.declare_dram_parameter("bn_mean", [COUT], F32, isOutput=False)
    var_d = nc.declare_dram_parameter("bn_var", [COUT], F32, isOutput=False)
    m2w_d = nc.declare_dram_parameter("move2_w", [COUT, E], F32, isOutput=False)
    m2b_d = nc.declare_dram_parameter("move2_b", [COUT], F32, isOutput=False)
    pa_d = nc.declare_dram_parameter("prelu_a", [COUT], F32, isOutput=False)
    m3w_d = nc.declare_dram_parameter("move3_w", [COUT, E], F32, isOutput=False)
    m3b_d = nc.declare_dram_parameter("move3_b", [COUT], F32, isOutput=False)
    y_d = nc.declare_dram_parameter("y", [B_LOC, COUT, H, W], F32, isOutput=True)

    hra_dram = nc.dram_tensor("hra_bounce", [COUT], F32)

    dbg = {}
    if DEBUG:
        dbg["bias1"] = nc.declare_dram_parameter("dbg_bias1", [128, 2, B_LOC], F32, isOutput=True)
        dbg["A"] = nc.declare_dram_parameter("dbg_A", [COUT, 1], F32, isOutput=True)
        dbg["C"] = nc.declare_dram_parameter("dbg_C", [COUT, B_LOC], F32, isOutput=True)
        dbg["C3"] = nc.declare_dram_parameter("dbg_C3", [COUT, B_LOC], F32, isOutput=True)
        dbg["p0"] = nc.declare_dram_parameter("dbg_p0", [2, 128, 128], F32, isOutput=True)
        dbg["wdr"] = nc.declare_dram_parameter("dbg_wdr", [128, 2, 9, 128], F32, isOutput=True)
        dbg["ad"] = nc.declare_dram_parameter("dbg_ad", [B_LOC, 128, 2, 1024], F32, isOutput=True)
        dbg["s_sb"] = nc.declare_dram_parameter("dbg_s", [128, 4, B_LOC], F32, isOutput=True)
        dbg["ps0"] = nc.declare_dram_parameter("dbg_ps0", [B_LOC, COUT, 512], F32, isOutput=True)

    with tile.TileContext(nc) as tc:
        _body(tc, nc, locals(), dbg)
    if split_waits:
        _split_multi_waits(nc)
    return nc


def _col(pool, nc, dram_vec, n=COUT, tag=None):
    """[n] DRAM vector -> [n, 1] per-partition column tile."""
    t = pool.tile([n, 1], F32, tag=tag or dram_vec.name + "_col")
    nc.sync.dma_start(out=t[:], in_=dram_vec[:].rearrange("(c one) -> c one", one=1))
    return t


def _body(tc, nc, d, dbg=None):
    x_d, emb_d, y_d = d["x_d"], d["emb_d"], d["y_d"]
    hra_dram = d["hra_dram"]

    from contextlib import ExitStack

    ctx = ExitStack()
    const = ctx.enter_context(tc.tile_pool(name="const", bufs=1))
    swt_pool = ctx.enter_context(tc.tile_pool(name="swt", bufs=4))
    xpad_pool = ctx.enter_context(tc.tile_pool(name="xpad", bufs=2))
    adr_pool = ctx.enter_context(tc.tile_pool(name="adr", bufs=2))
    out_pool = ctx.enter_context(tc.tile_pool(name="outsb", bufs=2))
    tmp_pool = ctx.enter_context(tc.tile_pool(name="tmp", bufs=4))
    dbg_pool = ctx.enter_context(tc.tile_pool(name="dbgpool", bufs=1)) if dbg else None
    ps_pre = ctx.enter_context(tc.tile_pool(name="ps_pre", bufs=2, space="PSUM"))
    ps_main = ctx.enter_context(tc.tile_pool(name="ps_main", bufs=6, space="PSUM"))

    # ---------------- parameter loads ----------------
    cw_sb = const.tile([COUT, CIN * 9], F32, tag="cw")
    nc.sync.dma_start(out=cw_sb[:], in_=d["cw_d"][:].rearrange("o i kh kw -> o (i kh kw)"))

    mw1 = const.tile([128, 2, E], F32, tag="mw1")  # [c_part, c_chunk, e]
    nc.sync.dma_start(out=mw1[:, 0, :], in_=d["m1w_d"][0:128, :])
    nc.sync.dma_start(out=mw1[:, 1, :], in_=d["m1w_d"][128:256, :])
    mw2 = const.tile([128, E], F32, tag="mw2")
    nc.sync.dma_start(out=mw2[:], in_=d["m2w_d"][:])
    mw3 = const.tile([128, E], F32, tag="mw3")
    nc.sync.dma_start(out=mw3[:], in_=d["m3w_d"][:])

    sT = const.tile([128, 4, B_LOC], F32, tag="sT")  # emb^T in 4 e-chunks
    for k in range(4):
        nc.sync.dma_start(out=sT[:, k, :],
                          in_=emb_d[:, k * 128:(k + 1) * 128].rearrange("b e -> e b"))

    cbc = _col(const, nc, d["cb_d"])
    gamc = _col(const, nc, d["gam_d"])
    betc = _col(const, nc, d["bet_d"])
    muc = _col(const, nc, d["mu_d"])
    varc = _col(const, nc, d["var_d"])
    m2bc = _col(const, nc, d["m2b_d"])
    m3bc = _col(const, nc, d["m3b_d"])
    pac = _col(const, nc, d["pa_d"])
    m1bc = const.tile([128, 2], F32, tag="m1b")
    nc.sync.dma_start(out=m1bc[:], in_=d["m1b_d"][:].rearrange("(j c) -> c j", c=128))

    # ---------------- scalar-engine precompute ----------------
    # inv = gamma / sqrt(var + 1e-5), via exp(-0.5 * ln(var + 1e-5))
    epsc = const.tile([COUT, 1], F32, tag="epsc")
    nc.vector.memset(epsc[:], 1e-5)
    lv = const.tile([COUT, 1], F32, tag="lv")
    nc.scalar.activation(lv[:], varc[:], AF.Ln, bias=epsc[:])
    rsq = const.tile([COUT, 1], F32, tag="rsq")
    nc.scalar.activation(rsq[:], lv[:], AF.Exp, scale=-0.5)
    inv = const.tile([COUT, 1], F32, tag="inv")
    nc.vector.tensor_mul(inv[:], rsq[:], gamc[:])

    # mean |conv_w| per output channel
    absw = const.tile([COUT, CIN * 9], F32, tag="absw")
    asum = const.tile([COUT, 1], F32, tag="asum")
    nc.scalar.activation(absw[:], cw_sb[:], AF.Abs, accum_out=asum[:])

    # sign(conv_w), reordered tap-major: sw_re[o, tap, j, i] = sign(cw[o, j*128+i, tap])
    sw_re = const.tile([COUT, 9, 2, 128], BF16, tag="swre")
    nc.scalar.activation(
        sw_re[:].rearrange("p t j i -> p (j i) t"),
        cw_sb[:].rearrange("p (i t) -> p i t", t=9),
        AF.Sign,
    )

    # silu(emb)^T = emb^T * sigmoid(emb^T)
    eneg = const.tile([128, 4, B_LOC], F32, tag="eneg")
    nc.scalar.activation(eneg[:], sT[:], AF.Exp, scale=-1.0)
    den = const.tile([128, 4, B_LOC], F32, tag="den")
    nc.vector.tensor_scalar_add(den[:], eneg[:], 1.0)
    rec = const.tile([128, 4, B_LOC], F32, tag="rec")
    nc.vector.reciprocal(rec[:], den[:])
    s_sb = const.tile([128, 4, B_LOC], F32, tag="s_sb")
    nc.vector.tensor_mul(s_sb[:], rec[:], sT[:])

    # ---------------- identity + weight transposes ----------------
    ident = const.tile([128, 128], F32, tag="ident")
    nc.vector.memset(ident[:], 1.0)
    nc.gpsimd.affine_select(
        ident[:], ident[:], pattern=[[-1, 128]], base=0,
        channel_multiplier=1, compare_op=ALU.is_equal, fill=0.0,
    )

    m1T = const.tile([128, 8, 128], F32, tag="m1T")  # [(c*4+k), :]
    m2T = const.tile([128, 4, 128], F32, tag="m2T")
    m3T = const.tile([128, 4, 128], F32, tag="m3T")
    for c in range(2):
        for k in range(4):
            pst = ps_pre.tile([128, 128], F32, tag="pre")
            nc.tensor.transpose(pst[:], mw1[:, c, k * 128:(k + 1) * 128], ident[:])
            nc.vector.tensor_copy(m1T[:, c * 4 + k, :], pst[:])
    for k in range(4):
        pst = ps_pre.tile([128, 128], F32, tag="pre")
        nc.tensor.transpose(pst[:], mw2[:, k * 128:(k + 1) * 128], ident[:])
        nc.vector.tensor_copy(m2T[:, k, :], pst[:])
    for k in range(4):
        pst = ps_pre.tile([128, 128], F32, tag="pre")
        nc.tensor.transpose(pst[:], mw3[:, k * 128:(k + 1) * 128], ident[:])
        nc.vector.tensor_copy(m3T[:, k, :], pst[:])

    # ---------------- per-channel affine constants ----------------
    A = const.tile([COUT, 1], F32, tag="A")
    nc.vector.tensor_scalar(A[:], asum[:], 1.0 / 2304.0, None, op0=ALU.mult)
    nc.vector.tensor_mul(A[:], A[:], inv[:])
    nc.vector.tensor_scalar(A[:], A[:], 1e-12, None, op0=ALU.max)

    t0 = const.tile([COUT, 1], F32, tag="t0")
    nc.vector.tensor_sub(t0[:], cbc[:], muc[:])
    nc.vector.tensor_mul(t0[:], t0[:], inv[:])
    nc.vector.tensor_add(t0[:], t0[:], betc[:])
    nc.vector.tensor_add(t0[:], t0[:], m2bc[:])

    alc = const.tile([COUT, 1], F32, tag="alc")
    nc.vector.tensor_scalar(alc[:], pac[:], 0.5, 0.5, op0=ALU.mult, op1=ALU.add)
    bec = const.tile([COUT, 1], F32, tag="bec")
    nc.vector.tensor_scalar(bec[:], pac[:], -0.5, 0.5, op0=ALU.mult, op1=ALU.add)
    aA = const.tile([COUT, 1], F32, tag="aA")
    nc.vector.tensor_mul(aA[:], alc[:], A[:])
    bA = const.tile([COUT, 1], F32, tag="bA")
    nc.vector.tensor_mul(bA[:], bec[:], A[:])

    # P0[j][p, m] = 0.5/A[m] where (j*128 + p) in {2m, 2m+1}.
    # Built transposed (P0T[m, pg] with per-partition value 0.5/A[m]) so no
    # cross-partition broadcast is needed, then PE-transposed per 128-block.
    hrac = const.tile([COUT, 1], F32, tag="hrac")
    nc.vector.reciprocal(hrac[:], A[:])
    nc.vector.tensor_scalar(hrac[:], hrac[:], 0.5, None, op0=ALU.mult)
    zt = const.tile([128, 256], F32, tag="zt")
    nc.vector.memset(zt[:], 0.0)
    p0T = const.tile([128, 256], F32, tag="p0T")
    nc.vector.tensor_scalar(p0T[:], zt[:], hrac[:], None, op0=ALU.add)
    nc.gpsimd.affine_select(
        p0T[:], p0T[:], pattern=[[1, 256]], base=0,
        channel_multiplier=-2, compare_op=ALU.is_ge, fill=0.0,
    )
    nc.gpsimd.affine_select(
        p0T[:], p0T[:], pattern=[[-1, 256]], base=1,
        channel_multiplier=2, compare_op=ALU.is_ge, fill=0.0,
    )
    p0 = []
    for j in range(2):
        psP = ps_pre.tile([128, 128], F32, tag="pre", name=f"psP_{j}")
        nc.tensor.transpose(psP[:], p0T[:, j * 128:(j + 1) * 128], ident[:])
        pj = const.tile([128, 128], F32, tag=f"p0_{j}", name=f"p0j_{j}")
        nc.vector.tensor_copy(pj[:], psP[:])
        p0.append(pj)

    # ---------------- bias matmuls (contract over e) ----------------
    bias1 = const.tile([128, 2, B_LOC], F32, tag="bias1")  # [c, j, b]
    for j in range(2):
        psb = ps_pre.tile([128, B_LOC], F32, tag="pre")
        for k in range(4):
            nc.tensor.matmul(psb[:], m1T[:, j * 4 + k, :], s_sb[:, k, :],
                             start=(k == 0), stop=(k == 3))
        nc.vector.tensor_scalar(bias1[:, j, :], psb[:], m1bc[:, j:j + 1], None,
                                op0=ALU.add)
    psb2 = ps_pre.tile([128, B_LOC], F32, tag="pre")
    for k in range(4):
        nc.tensor.matmul(psb2[:], m2T[:, k, :], s_sb[:, k, :],
                         start=(k == 0), stop=(k == 3))
    C = const.tile([COUT, B_LOC], F32, tag="C")
    nc.vector.tensor_scalar(C[:], psb2[:], t0[:], None, op0=ALU.add)

    psb3 = ps_pre.tile([128, B_LOC], F32, tag="pre")
    for k in range(4):
        nc.tensor.matmul(psb3[:], m3T[:, k, :], s_sb[:, k, :],
                         start=(k == 0), stop=(k == 3))
    C3 = const.tile([COUT, B_LOC], F32, tag="C3")
    nc.vector.tensor_scalar(C3[:], psb3[:], m3bc[:], None, op0=ALU.add)

    aCC3 = const.tile([COUT, B_LOC], F32, tag="aCC3")
    nc.vector.tensor_scalar(aCC3[:], C[:], alc[:], None, op0=ALU.mult)
    nc.vector.tensor_add(aCC3[:], aCC3[:], C3[:])
    bC = const.tile([COUT, B_LOC], F32, tag="bC")
    nc.vector.tensor_scalar(bC[:], C[:], bec[:], None, op0=ALU.mult)

    # ---------------- conv weights: transpose to [i, o] fp8 blocks ----------
    w_dr = const.tile([128, 2, 9, 128], F8, tag="wdr")  # [i, j, tap, o]
    for t in range(9):
        for j in range(2):
            swt = swt_pool.tile([128, 128], BF16, tag="swt")
            nc.sync.dma_start_transpose(swt[:], sw_re[:, t, j, :])
            nc.vector.tensor_copy(w_dr[:, j, t, :], swt[:])

    if dbg:
        dbgcast = dbg_pool.tile([128, 2, 9, 128], F32, tag="dbgcast")
        nc.vector.tensor_copy(dbgcast[:], w_dr[:])
        nc.sync.dma_start(out=dbg["wdr"][:], in_=dbgcast[:])
        nc.sync.dma_start(out=dbg["A"][:], in_=A[:])
        nc.sync.dma_start(out=dbg["C"][:], in_=C[:])
        nc.sync.dma_start(out=dbg["C3"][:], in_=C3[:])
        nc.sync.dma_start(out=dbg["bias1"][:], in_=bias1[:])
        nc.sync.dma_start(out=dbg["s_sb"][:], in_=s_sb[:])
        for j in range(2):
            nc.sync.dma_start(out=dbg["p0"][j], in_=p0[j][:])

    # ---------------- main loop ----------------
    for b in range(B_LOC):
        xp = xpad_pool.tile([128, 2, JSTR], F32, tag="xp")
        ad = adr_pool.tile([128, 2, JSTR], F8, tag="ad")
        # zero the one-cell borders of the 66x66 padded layout
        for t_, tile_ in ((0, xp), (1, ad)):
            nc.gpsimd.memset(tile_[:, :, 0:PW], 0.0)
            nc.gpsimd.memset(tile_[:, :, NPAD - PW:NPAD], 0.0)
            cols = tile_[:, :, 0:NPAD].rearrange("p j (r v) -> p j r v", v=PW)
            nc.gpsimd.memset(cols[:, :, :, 0:1], 0.0)
            nc.gpsimd.memset(cols[:, :, :, PW - 1:PW], 0.0)

        for j in range(2):
            interior = (
                xp[:, j, PW + 1:PW + 1 + H * PW]
                .rearrange("p (h w) -> p h w", w=PW)[:, :, 0:W]
            )
            nc.sync.dma_start(out=interior, in_=x_d[b, j * 128:(j + 1) * 128, :, :])

        for j in range(2):
            xin = (
                xp[:, j, PW + 1:PW + 1 + H * PW]
                .rearrange("p (h w) -> p h w", w=PW)[:, :, 0:W]
            )
            aout = (
                ad[:, j, PW + 1:PW + 1 + H * PW]
                .rearrange("p (h w) -> p h w", w=PW)[:, :, 0:W]
            )
            nc.scalar.activation(aout, xin, AF.Sign, bias=bias1[:, j, b:b + 1])

        osb = out_pool.tile([128, 4224], F32, tag="osb")
        for grp in GROUPS:
            pss = {}
            for ci in grp:
                pss[ci] = ps_main.tile([128, 512], F32, tag="ps", name=f"ps_{b}_{ci}")
            for t in range(9):
                off = (t // 3) * PW + (t % 3)
                for ci in grp:
                    q0, L = CHUNKS[ci]
                    nc.tensor.matmul(
                        pss[ci][:, 0:L], w_dr[:, :, t, :],
                        ad[:, :, q0 + off:q0 + off + L],
                        start=(t == 0), stop=False,
                        perf_mode=DR, skip_group_check=True,
                    )
            for j2 in range(2):
                for ci in grp:
                    q0, L = CHUNKS[ci]
                    nc.tensor.matmul(
                        pss[ci][:, 0:L], p0[j2][:],
                        xp[:, j2, PW + 1 + q0:PW + 1 + q0 + L],
                        start=False, stop=(j2 == 1), skip_group_check=True,
                    )
            if dbg and grp[0] == 0:
                dbga = dbg_pool.tile([128, 2, 1024], F32, tag="dbga")
                nc.vector.tensor_copy(dbga[:], ad[:, :, 0:1024])
                nc.sync.dma_start(out=dbg["ad"][b], in_=dbga[:])
                dbgp = dbg_pool.tile([128, 512], F32, tag="dbgp")
                nc.vector.tensor_copy(dbgp[:], pss[0][:])
                nc.sync.dma_start(out=dbg["ps0"][b], in_=dbgp[:])
            for ci in grp:
                q0, L = CHUNKS[ci]
                if USE_PRELU:
                    tt = tmp_pool.tile([128, 512], F32, tag="tt")
                    nc.scalar.activation(tt[:, 0:L], pss[ci][:, 0:L], AF.Prelu,
                                         bias=C[:, b:b + 1], scale=A[:], alpha=pac[:])
                    nc.vector.tensor_scalar(osb[:, q0:q0 + L], tt[:, 0:L],
                                            C3[:, b:b + 1], None, op0=ALU.add)
                else:
                    tt = tmp_pool.tile([128, 512], F32, tag="tt")
                    nc.scalar.activation(tt[:, 0:L], pss[ci][:, 0:L], AF.Abs,
                                         bias=bC[:, b:b + 1], scale=bA[:])
                    ut = tmp_pool.tile([128, 512], F32, tag="ut")
                    nc.vector.tensor_scalar(ut[:, 0:L], pss[ci][:, 0:L], aA[:],
                                            aCC3[:, b:b + 1], op0=ALU.mult, op1=ALU.add)
                    nc.vector.tensor_add(osb[:, q0:q0 + L], ut[:, 0:L], tt[:, 0:L])

        osrc = (
            osb[:, 0:H * PW]
            .rearrange("p (h w) -> p h w", w=PW)[:, :, 0:W]
        )
        nc.sync.dma_start(out=y_d[b, :, :, :], in_=osrc)

    ctx.close()


_cached_nc = None


def _get_nc():
    global _cached_nc
    if _cached_nc is None:
        _cached_nc = build_program()
    return _cached_nc


def kernel(**inputs):
    from concourse.bass_utils import run_bass_kernel_spmd

    nc = _get_nc()
    full = {k: np.ascontiguousarray(np.asarray(v, dtype=np.float32))
            for k, v in inputs.items()}
    in_maps = []
    for c in range(N_CORES):
        m = dict(full)
        m["x"] = full["x"][c * B_LOC:(c + 1) * B_LOC]
        m["emb"] = full["emb"][c * B_LOC:(c + 1) * B_LOC]
        in_maps.append(m)
    res = run_bass_kernel_spmd(nc, in_maps, list(range(N_CORES)))
    return np.concatenate([res.results[c]["y"] for c in range(N_CORES)], axis=0)
